# revision 28
# baseline (speedup 1.0000x reference)
"""Trainium2 Bass kernel for nn_MultiHeadRelativeAttention (S=256, E=1024, H=16).

Sharding: tensor-parallel over heads. Each of 8 cores owns 2 heads (a 128-wide
d-slice), computes its projections, scores, softmax, attention and a partial
output projection; the host sums the 8 partials (the Wo contraction over d is
the all-reduce).

Design (v15 = best-measured v2 structure + surgical fixes):
  - all inputs host-prelaid in exact SBUF tile layout (contiguous DMAs).
  - a2 with both heads per matmul: 128-partition contraction, one N=256
    matmul per query row. Stationary is a 128-col window (stride 127) into a
    zero-spaced buffer B holding nq[h0,i] at col 128w (partitions 0-63) and
    nq[h1,i] at col 128w+64 (partitions 64-127); PSUM rows 0-63 are head-0
    scores, 64-127 head-1. 4 groups of 64 rows, one PSUM bank each. The
    modest per-matmul DMA demand (~354 GB/s) keeps the stream PE-paced and
    the p-state pinned high.
  - a1 folded into the score accumulation: two K=64 matmuls open each group.
  - a3 (Transformer-XL rel-shift) via a consolidated bf16 DRAM bounce, then
    folded into the score PSUM with identity-stationary matmuls closing each
    group.
  - single B buffer: half-memsets on DVE at startup, live-column rewrites
    per group pipelined mid-stream.
  - DMA rings: sync = critical consts + s_k chunks + bounce; scalar carries
    the output DMAs (off the chunk-generation path); gpsimd = secondary
    consts.
  - PE warmup transposes against the identity climb the p-state ramp during
    the initial DMA window.
"""

import sys

if "/opt/trn_rl_repo" not in sys.path:
    sys.path.insert(0, "/opt/trn_rl_repo")

import numpy as np

import concourse.bass as bass
import concourse.mybir as mybir
import concourse.tile as tile
from concourse import bacc
from concourse.masks import make_identity

S = 256
E = 1024
H = 16
HD = 64
NCORES = 8
DHB = 128
SCALING = float(HD) ** -0.5

F32 = mybir.dt.float32
BF16 = mybir.dt.bfloat16
NPBF = np.dtype("bfloat16")

NCHUNK = 16
CHI = 16


def emit(tc: tile.TileContext, t: dict):
    nc = tc.nc
    from contextlib import ExitStack

    ctx = ExitStack()
    const = ctx.enter_context(tc.tile_pool(name="const", bufs=1))
    skp = ctx.enter_context(tc.tile_pool(name="skp", bufs=6))
    work = ctx.enter_context(tc.tile_pool(name="work", bufs=3))
    psS = ctx.enter_context(tc.tile_pool(name="psS", bufs=3, space="PSUM"))
    psT = ctx.enter_context(tc.tile_pool(name="psT", bufs=2, space="PSUM"))
    psM = ctx.enter_context(tc.tile_pool(name="psM", bufs=2, space="PSUM"))

    ident = const.tile([128, 128], BF16, tag="ident")
    make_identity(nc, ident)

    # ---- critical consts + s_k stream on the Sync ring ----
    sq = const.tile([128, 1], F32, tag="sq")
    wq = const.tile([128, 8, 128], BF16, tag="wq")
    xq = const.tile([128, 8, 256], BF16, tag="xq")
    for sb, name in ((sq, "sq"), (wq, "wq"), (xq, "xq")):
        nc.sync.dma_start(out=sb, in_=t[name])

    skq = {}

    def load_chunk(k):
        st = skp.tile([128, CHI, 256], BF16, tag="skt", name=f"skt{k}")
        nc.sync.dma_start(out=st, in_=t["skT"][:, 4096 * k:4096 * (k + 1)])
        skq[k] = st

    for k in range(3):
        load_chunk(k)

    # ---- secondary consts on the gpsimd (Pool/SWDGE) ring ----
    wk = const.tile([128, 8, 128], BF16, tag="wk")
    xk = const.tile([128, 8, 256], BF16, tag="xk")
    wr = const.tile([128, 8, 128], BF16, tag="wr")
    xp = const.tile([128, 8, 256], BF16, tag="xp")
    wv = const.tile([128, 8, 128], BF16, tag="wv")
    xv = const.tile([128, 8, 256], BF16, tag="xv")
    wo = const.tile([128, 1024], BF16, tag="wo")
    mnot = const.tile([128, 4, 256], BF16, tag="mnot")
    for sb, name in ((wk, "wk"), (xk, "xk"), (wr, "wr"), (xp, "xp"),
                     (wv, "wv"), (xv, "xv"), (wo, "wo"), (mnot, "mnot")):
        nc.gpsimd.dma_start(out=sb, in_=t[name])

    # zero pads of the a3 bounce scratch
    zt4 = const.tile([128, 4, 256], BF16, tag="zt4")
    nc.gpsimd.memset(zt4, 0.0)
    nc.gpsimd.dma_start(
        out=bass.AP(tensor=t["a3scr"].tensor, offset=t["a3scr"].offset + 256,
                    ap=[[512, 128], [65536, 4], [1, 256]]),
        in_=zt4)

    # PE warmup on the identity while DMAs land
    warm = psM.tile([128, 128], BF16, tag="pm", name="warm")
    for _ in range(14):
        nc.tensor.matmul(warm, ident, ident, start=True, stop=True,
                         is_transpose=True, skip_group_check=True)

    # B buffer: zero-spaced stationary, half-memsets on DVE
    B = const.tile([128, 8192], BF16, tag="B")
    Bv = B.rearrange("p (w c) -> p w c", c=128)
    nc.vector.memset(B[:, 0:4096], 0.0)

    # ---- projections (transposed): (128 d, 256 s) ----
    def proj_T(wsb, xsb, name):
        ps = psM.tile([128, 512], F32, tag="pm", name=name)
        for c in range(8):
            nc.tensor.matmul(ps[:, 0:256], wsb[:, c, :], xsb[:, c, :],
                             start=(c == 0), stop=(c == 7))
        return ps

    nqT = const.tile([128, 256], BF16, tag="nqT")
    nc.vector.tensor_scalar_add(out=nqT, in0=proj_T(wq, xq, "ps_nq")[:, 0:256],
                                scalar1=sq)
    nc.vector.memset(B[:, 4096:8192], 0.0)

    keyT = const.tile([128, 256], BF16, tag="keyT")
    nc.scalar.copy(out=keyT, in_=proj_T(wk, xk, "ps_key")[:, 0:256])

    def scatter_a(g):
        """live cols for windows w<32 of group g."""
        base = 64 * g
        nc.gpsimd.tensor_copy(out=Bv[0:64, 0:32, 0],
                              in_=nqT[0:64, base:base + 32])
        nc.gpsimd.tensor_copy(out=Bv[64:128, 0:32, 64],
                              in_=nqT[64:128, base:base + 32])

    def scatter_b(g):
        base = 64 * g
        nc.gpsimd.tensor_copy(out=Bv[0:64, 32:64, 0],
                              in_=nqT[0:64, base + 32:base + 64])
        nc.gpsimd.tensor_copy(out=Bv[64:128, 32:64, 64],
                              in_=nqT[64:128, base + 32:base + 64])

    S_ps = [None] * 4

    def open_group(g):
        base = 64 * g
        sp = psS.tile([128, 512], F32, tag="S", name=f"S{g}")
        S_ps[g] = sp
        for h in range(2):
            hsl = slice(64 * h, 64 * h + 64)
            nc.tensor.matmul(sp[hsl, 0:256], nqT[hsl, base:base + 64],
                             keyT[hsl, :], start=True, stop=False,
                             skip_group_check=True)

    def a2_run(g, w0, w1):
        base = 64 * g
        sp = S_ps[g]
        for w in range(w0, w1):
            i = base + w
            k, r = divmod(i, CHI)
            st = skq[k]
            nc.tensor.matmul(sp[:, 0:256], B[:, 127 * w:127 * w + 128],
                             st[:, r, :], start=False, stop=False,
                             skip_group_check=True)
            if r == CHI - 1:
                del skq[k]
                if k + 3 < NCHUNK:
                    load_chunk(k + 3)

    a3sb = None

    def a3_inject(g):
        """fold shifted a3 into the score PSUM; closes the accumulation."""
        sp = S_ps[g]
        qsl = slice(64 * (g % 2), 64 * (g % 2) + 64)
        for h in range(2):
            nc.tensor.matmul(sp[64 * h:64 * h + 64, 0:256],
                             ident[qsl, qsl], a3sb[qsl, h, g // 2, :],
                             start=False, stop=(h == 1),
                             skip_group_check=True)

    def a3_emit(relT):
        nonlocal a3sb
        raw4 = work.tile([128, 4, 256], BF16, tag="raw4")
        for h in range(2):
            hsl = slice(64 * h, 64 * h + 64)
            for ib in range(2):
                ps = psM.tile([128, 512], F32, tag="pm", name=f"a3r{h}{ib}")
                nc.tensor.matmul(ps[:, 0:256],
                                 nqT[hsl, 128 * ib:128 * ib + 128],
                                 relT[hsl, :], start=True, stop=True)
                nc.scalar.copy(out=raw4[:, 2 * h + ib, :], in_=ps[:, 0:256])
        scr = t["a3scr"]
        nc.scalar.dma_start(
            out=bass.AP(tensor=scr.tensor, offset=scr.offset,
                        ap=[[512, 128], [65536, 4], [1, 256]]),
            in_=raw4)
        sh = const.tile([128, 2, 2, 256], BF16, tag="a3sb")
        for h in range(2):
            nc.scalar.dma_start(
                out=sh[:, h, :, :],
                in_=bass.AP(tensor=scr.tensor,
                            offset=scr.offset + 255 + 131072 * h,
                            ap=[[511, 128], [65408, 2], [1, 256]]))
        a3sb = sh

    # ---- per-group tail ----
    value = [None, None]
    sc_t = [None] * 4
    stc_t = [None] * 4
    aT_t = [None] * 4

    def tail_vec(g):
        sp = S_ps[g]
        w3 = work.tile([128, 256], F32, tag="w3")
        nc.vector.tensor_mul(out=w3, in0=sp[:, 0:256], in1=mnot[:, g, :])
        ex = work.tile([128, 256], F32, tag="ex")
        nc.scalar.activation(out=ex, in_=w3,
                             func=mybir.ActivationFunctionType.Exp, scale=1.0)
        den = work.tile([128, 1], F32, tag="den")
        nc.vector.reduce_sum(out=den, in_=ex, axis=mybir.AxisListType.X)
        rden = work.tile([128, 1], F32, tag="rden")
        nc.vector.reciprocal(out=rden, in_=den)
        sc = work.tile([128, 256], BF16, tag="sc", name=f"sc{g}")
        nc.vector.tensor_scalar_mul(out=sc, in0=ex, scalar1=rden)
        sc_t[g] = sc

    def tail_tr(g):
        sc = sc_t[g]
        stc = []
        for jb in range(2):
            tp = psT.tile([128, 128], BF16, tag="tp", name=f"tp{g}{jb}")
            nc.tensor.transpose(tp, sc[:, 128 * jb:128 * jb + 128], ident)
            st = work.tile([128, 128], BF16, tag="stc", name=f"stc{g}{jb}")
            if jb == 0:
                nc.vector.tensor_copy(out=st, in_=tp)
            else:
                nc.scalar.copy(out=st, in_=tp)
            stc.append(st)
        stc_t[g] = stc

    def tail_av(g):
        stc = stc_t[g]
        av = psM.tile([128, 512], F32, tag="pm", name=f"av{g}")
        for h in range(2):
            hsl = slice(64 * h, 64 * h + 64)
            for jb in range(2):
                nc.tensor.matmul(av[hsl, 0:64], value[jb][:, hsl],
                                 stc[jb][:, hsl], start=(jb == 0),
                                 stop=(jb == 1), skip_group_check=True)
        aT = work.tile([128, 64], BF16, tag="aT", name=f"aT{g}")
        nc.scalar.copy(out=aT, in_=av[:, 0:64])
        aT_t[g] = aT

    def tail_out(g):
        aT = aT_t[g]
        ob = work.tile([64, 1024], F32, tag="ob", name=f"ob{g}")
        for eh in range(2):
            op = psM.tile([128, 512], F32, tag="pm", name=f"op{g}{eh}")
            nc.tensor.matmul(op[0:64, :], aT, wo[:, 512 * eh:512 * (eh + 1)],
                             start=True, stop=True, skip_group_check=True)
            if eh == 0:
                nc.vector.tensor_copy(out=ob[:, 0:512], in_=op[0:64, :])
            else:
                nc.scalar.copy(out=ob[:, 512:1024], in_=op[0:64, :])
        nc.scalar.dma_start(out=t["outp"][64 * g:64 * g + 64, :], in_=ob)

    # ---- schedule ----
    scatter_a(0)
    open_group(0)
    a2_run(0, 0, 16)

    relT = const.tile([128, 256], BF16, tag="relT")
    nc.scalar.copy(out=relT, in_=proj_T(wr, xp, "ps_rel")[:, 0:256])

    a2_run(0, 16, 32)
    scatter_b(0)
    a3_emit(relT)

    a2_run(0, 32, 48)

    for jb in range(2):
        ps = psM.tile([128, 512], F32, tag="pm", name=f"ps_val{jb}")
        for c in range(8):
            nc.tensor.matmul(ps[:, 0:128], xv[:, c, 128 * jb:128 * jb + 128],
                             wv[:, c, :], start=(c == 0), stop=(c == 7))
        vsb = const.tile([128, 128], BF16, tag=f"value{jb}")
        nc.scalar.copy(out=vsb, in_=ps[:, 0:128])
        value[jb] = vsb

    scatter_a(1)
    a2_run(0, 48, 64)

    open_group(1)
    a2_run(1, 0, 16)
    scatter_b(1)
    a2_run(1, 16, 32)
    scatter_a(2)
    a2_run(1, 32, 64)
    a3_inject(0)
    tail_vec(0)
    a3_inject(1)
    tail_vec(1)

    open_group(2)
    a2_run(2, 0, 16)
    scatter_b(2)
    a2_run(2, 16, 32)
    tail_tr(0)
    scatter_a(3)
    a2_run(2, 32, 48)
    tail_av(0)
    a2_run(2, 48, 64)
    a3_inject(2)
    tail_vec(2)
    tail_out(0)

    open_group(3)
    a2_run(3, 0, 16)
    scatter_b(3)
    a2_run(3, 16, 32)
    tail_tr(1)
    a2_run(3, 32, 40)
    tail_av(1)
    a2_run(3, 40, 48)
    tail_out(1)
    a2_run(3, 48, 64)
    a3_inject(3)
    tail_vec(3)
    tail_tr(2)
    tail_av(2)
    tail_out(2)

    tail_tr(3)
    tail_av(3)
    tail_out(3)

    ctx.close()


def build():
    nc = bacc.Bacc("TRN2", target_bir_lowering=False, debug=False)
    t = {}

    def inp(name, shape, dt=BF16):
        t[name] = nc.dram_tensor(name, list(shape), dt,
                                 kind="ExternalInput").ap()

    inp("skT", (128, S * S))
    inp("sq", (128, 1), F32)
    for n in ("xq", "xk", "xp", "xv"):
        inp(n, (128, 8, 256))
    for n in ("wq", "wk", "wr", "wv"):
        inp(n, (128, 8, 128))
    inp("wo", (128, 1024))
    inp("mnot", (128, 4, 256))
    t["a3scr"] = nc.dram_tensor("a3scr", [4, 128, 512], BF16).ap()
    t["outp"] = nc.dram_tensor("outp", [S, E], F32, kind="ExternalOutput").ap()

    with tile.TileContext(nc) as tc:
        emit(tc, t)
    nc.compile()
    return nc


def make_in_maps(inputs: dict) -> list[dict]:
    q = np.asarray(inputs["q"], np.float32)
    k = np.asarray(inputs["k"], np.float32)
    v = np.asarray(inputs["v"], np.float32)
    p = np.asarray(inputs["p"], np.float32)
    mask = np.asarray(inputs["mask"])
    s_q = np.asarray(inputs["s_q"], np.float32)
    s_k = np.asarray(inputs["s_k"], np.float32)
    Wq = np.asarray(inputs["Wq"], np.float32)
    Wk = np.asarray(inputs["Wk"], np.float32)
    Wv = np.asarray(inputs["Wv"], np.float32)
    Wr = np.asarray(inputs["Wr"], np.float32)
    Wo = np.asarray(inputs["Wo"], np.float32)

    def actT(x):
        return np.ascontiguousarray(
            x.T.reshape(8, 128, 256).transpose(1, 0, 2)).astype(NPBF)

    xq, xk, xp, xv = actT(q), actT(k), actT(p), actT(v)

    maps = []
    for c in range(NCORES):
        rows = slice(c * DHB, (c + 1) * DHB)

        def wT(W):
            return np.ascontiguousarray(
                W[rows].T.reshape(8, 128, 128).transpose(1, 0, 2)
            ).astype(NPBF)

        skT = np.ascontiguousarray(s_k[:, rows].T).astype(NPBF)
        mn = np.empty((128, 4, 256), np.float32)
        for g in range(4):
            for h in range(2):
                mn[64 * h:64 * h + 64, g] = (
                    1.0 - mask[2 * c + h,
                               64 * g:64 * g + 64].astype(np.float32)
                ) * SCALING
        maps.append({
            "skT": skT,
            "xq": xq, "xk": xk, "xp": xp, "xv": xv,
            "wq": wT(Wq), "wk": wT(Wk), "wr": wT(Wr), "wv": wT(Wv),
            "wo": np.ascontiguousarray(Wo[:, rows].T).astype(NPBF),
            "sq": np.ascontiguousarray(s_q[0, rows][:, None]),
            "mnot": mn.astype(NPBF),
        })
    return maps


_NC = None


def kernel(**inputs) -> np.ndarray:
    global _NC
    from concourse.bass_utils import run_bass_kernel_spmd

    if _NC is None:
        _NC = build()
    maps = make_in_maps(inputs)
    res = run_bass_kernel_spmd(_NC, maps, list(range(NCORES)))
    out = np.zeros((S, E), np.float32)
    for r in res.results:
        out += np.asarray(r["outp"], np.float32)
    return out


if __name__ == "__main__":
    nc = build()
    print("build ok")


# revision 29
# speedup vs baseline: 1.2561x; 1.2561x over previous
"""Trainium2 Bass kernel for nn_MultiHeadRelativeAttention (S=256, E=1024, H=16).

Sharding: tensor-parallel over heads. Each of 8 cores owns 2 heads (a 128-wide
d-slice), computes its projections, scores, softmax, attention and a partial
output projection; the host sums the 8 partials (the Wo contraction over d is
the all-reduce).

Design (v15 = best-measured v2 structure + surgical fixes):
  - all inputs host-prelaid in exact SBUF tile layout (contiguous DMAs).
  - a2 with both heads per matmul: 128-partition contraction, one N=256
    matmul per query row. Stationary is a 128-col window (stride 127) into a
    zero-spaced buffer B holding nq[h0,i] at col 128w (partitions 0-63) and
    nq[h1,i] at col 128w+64 (partitions 64-127); PSUM rows 0-63 are head-0
    scores, 64-127 head-1. 4 groups of 64 rows, one PSUM bank each. The
    modest per-matmul DMA demand (~354 GB/s) keeps the stream PE-paced and
    the p-state pinned high.
  - a1 folded into the score accumulation: two K=64 matmuls open each group.
  - a3 (Transformer-XL rel-shift) via a consolidated bf16 DRAM bounce, then
    folded into the score PSUM with identity-stationary matmuls closing each
    group.
  - single B buffer: half-memsets on DVE at startup, live-column rewrites
    per group pipelined mid-stream.
  - DMA rings: sync = critical consts + s_k chunks + bounce; scalar carries
    the output DMAs (off the chunk-generation path); gpsimd = secondary
    consts.
  - PE warmup transposes against the identity climb the p-state ramp during
    the initial DMA window.
"""

import sys

if "/opt/trn_rl_repo" not in sys.path:
    sys.path.insert(0, "/opt/trn_rl_repo")

import numpy as np

import concourse.bass as bass
import concourse.mybir as mybir
import concourse.tile as tile
from concourse import bacc
from concourse.masks import make_identity

S = 256
E = 1024
H = 16
HD = 64
NCORES = 8
DHB = 128
SCALING = float(HD) ** -0.5

F32 = mybir.dt.float32
BF16 = mybir.dt.bfloat16
NPBF = np.dtype("bfloat16")

NCHUNK = 16
CHI = 16


def emit(tc: tile.TileContext, t: dict):
    nc = tc.nc
    from contextlib import ExitStack

    ctx = ExitStack()
    const = ctx.enter_context(tc.tile_pool(name="const", bufs=1))
    skp = ctx.enter_context(tc.tile_pool(name="skp", bufs=6))
    work = ctx.enter_context(tc.tile_pool(name="work", bufs=3))
    psS = ctx.enter_context(tc.tile_pool(name="psS", bufs=3, space="PSUM"))
    psT = ctx.enter_context(tc.tile_pool(name="psT", bufs=2, space="PSUM"))
    psM = ctx.enter_context(tc.tile_pool(name="psM", bufs=2, space="PSUM"))

    ident = const.tile([128, 128], BF16, tag="ident")
    make_identity(nc, ident)

    # ---- critical consts + s_k stream on the Sync ring ----
    sq = const.tile([128, 1], F32, tag="sq")
    wq = const.tile([128, 8, 128], BF16, tag="wq")
    xq = const.tile([128, 8, 256], BF16, tag="xq")
    wk = const.tile([128, 8, 128], BF16, tag="wk")
    xk = const.tile([128, 8, 256], BF16, tag="xk")
    wr = const.tile([128, 8, 128], BF16, tag="wr")
    xp = const.tile([128, 8, 256], BF16, tag="xp")
    for sb, name in ((sq, "sq"), (wq, "wq"), (xq, "xq"), (wk, "wk"),
                     (xk, "xk"), (wr, "wr"), (xp, "xp")):
        nc.sync.dma_start(out=sb, in_=t[name])

    skq = {}

    def load_chunk(k):
        st = skp.tile([128, CHI, 256], BF16, tag="skt", name=f"skt{k}")
        nc.sync.dma_start(out=st, in_=t["skT"][:, 4096 * k:4096 * (k + 1)])
        skq[k] = st

    for k in range(3):
        load_chunk(k)

    # ---- secondary consts on the gpsimd (Pool/SWDGE) ring ----
    wv = const.tile([128, 8, 128], BF16, tag="wv")
    xv = const.tile([128, 8, 256], BF16, tag="xv")
    wo = const.tile([128, 1024], BF16, tag="wo")
    mnot = const.tile([128, 4, 256], BF16, tag="mnot")
    for sb, name in ((wv, "wv"), (xv, "xv"), (wo, "wo"), (mnot, "mnot")):
        nc.gpsimd.dma_start(out=sb, in_=t[name])

    # zero pads of the a3 bounce scratch
    zt4 = const.tile([128, 4, 256], BF16, tag="zt4")
    nc.gpsimd.memset(zt4, 0.0)
    nc.gpsimd.dma_start(
        out=bass.AP(tensor=t["a3scr"].tensor, offset=t["a3scr"].offset + 256,
                    ap=[[512, 128], [65536, 4], [1, 256]]),
        in_=zt4)

    # PE warmup on the identity while DMAs land
    warm = psM.tile([128, 128], BF16, tag="pm", name="warm")
    for _ in range(14):
        nc.tensor.matmul(warm, ident, ident, start=True, stop=True,
                         is_transpose=True, skip_group_check=True)

    # B buffer: zero-spaced stationary, half-memsets on DVE
    B = const.tile([128, 8192], BF16, tag="B")
    Bv = B.rearrange("p (w c) -> p w c", c=128)
    nc.vector.memset(B[:, 0:4096], 0.0)

    # ---- projections (transposed): (128 d, 256 s) ----
    def proj_T(wsb, xsb, name):
        ps = psM.tile([128, 512], F32, tag="pm", name=name)
        for c in range(8):
            nc.tensor.matmul(ps[:, 0:256], wsb[:, c, :], xsb[:, c, :],
                             start=(c == 0), stop=(c == 7))
        return ps

    nqT = const.tile([128, 256], BF16, tag="nqT")
    nc.vector.tensor_scalar_add(out=nqT, in0=proj_T(wq, xq, "ps_nq")[:, 0:256],
                                scalar1=sq)
    nc.vector.memset(B[:, 4096:8192], 0.0)

    keyT = const.tile([128, 256], BF16, tag="keyT")
    nc.scalar.copy(out=keyT, in_=proj_T(wk, xk, "ps_key")[:, 0:256])

    def scatter_a(g):
        """live cols for windows w<32 of group g."""
        base = 64 * g
        nc.gpsimd.tensor_copy(out=Bv[0:64, 0:32, 0],
                              in_=nqT[0:64, base:base + 32])
        nc.gpsimd.tensor_copy(out=Bv[64:128, 0:32, 64],
                              in_=nqT[64:128, base:base + 32])

    def scatter_b(g):
        base = 64 * g
        nc.gpsimd.tensor_copy(out=Bv[0:64, 32:64, 0],
                              in_=nqT[0:64, base + 32:base + 64])
        nc.gpsimd.tensor_copy(out=Bv[64:128, 32:64, 64],
                              in_=nqT[64:128, base + 32:base + 64])

    S_ps = [None] * 4

    def open_group(g):
        base = 64 * g
        sp = psS.tile([128, 512], F32, tag="S", name=f"S{g}")
        S_ps[g] = sp
        for h in range(2):
            hsl = slice(64 * h, 64 * h + 64)
            nc.tensor.matmul(sp[hsl, 0:256], nqT[hsl, base:base + 64],
                             keyT[hsl, :], start=True, stop=False,
                             skip_group_check=True)

    def a2_run(g, w0, w1):
        base = 64 * g
        sp = S_ps[g]
        for w in range(w0, w1):
            i = base + w
            k, r = divmod(i, CHI)
            st = skq[k]
            nc.tensor.matmul(sp[:, 0:256], B[:, 127 * w:127 * w + 128],
                             st[:, r, :], start=False, stop=False,
                             skip_group_check=True)
            if r == CHI - 1:
                del skq[k]
                if k + 3 < NCHUNK:
                    load_chunk(k + 3)

    a3sb = None

    def a3_inject(g):
        """fold shifted a3 into the score PSUM; closes the accumulation."""
        sp = S_ps[g]
        qsl = slice(64 * (g % 2), 64 * (g % 2) + 64)
        for h in range(2):
            nc.tensor.matmul(sp[64 * h:64 * h + 64, 0:256],
                             ident[qsl, qsl], a3sb[qsl, h, g // 2, :],
                             start=False, stop=(h == 1),
                             skip_group_check=True)

    def a3_emit(relT):
        nonlocal a3sb
        raw4 = work.tile([128, 4, 256], BF16, tag="raw4")
        for h in range(2):
            hsl = slice(64 * h, 64 * h + 64)
            for ib in range(2):
                ps = psM.tile([128, 512], F32, tag="pm", name=f"a3r{h}{ib}")
                nc.tensor.matmul(ps[:, 0:256],
                                 nqT[hsl, 128 * ib:128 * ib + 128],
                                 relT[hsl, :], start=True, stop=True)
                nc.scalar.copy(out=raw4[:, 2 * h + ib, :], in_=ps[:, 0:256])
        scr = t["a3scr"]
        nc.sync.dma_start(
            out=bass.AP(tensor=scr.tensor, offset=scr.offset,
                        ap=[[512, 128], [65536, 4], [1, 256]]),
            in_=raw4)
        sh = const.tile([128, 2, 2, 256], BF16, tag="a3sb")
        for h in range(2):
            nc.sync.dma_start(
                out=sh[:, h, :, :],
                in_=bass.AP(tensor=scr.tensor,
                            offset=scr.offset + 255 + 131072 * h,
                            ap=[[511, 128], [65408, 2], [1, 256]]))
        a3sb = sh

    # ---- per-group tail ----
    value = [None, None]
    sc_t = [None] * 4
    stc_t = [None] * 4
    aT_t = [None] * 4

    def tail_vec(g):
        sp = S_ps[g]
        w3 = work.tile([128, 256], F32, tag="w3")
        nc.vector.tensor_mul(out=w3, in0=sp[:, 0:256], in1=mnot[:, g, :])
        ex = work.tile([128, 256], F32, tag="ex")
        nc.scalar.activation(out=ex, in_=w3,
                             func=mybir.ActivationFunctionType.Exp, scale=1.0)
        den = work.tile([128, 1], F32, tag="den")
        nc.vector.reduce_sum(out=den, in_=ex, axis=mybir.AxisListType.X)
        rden = work.tile([128, 1], F32, tag="rden")
        nc.vector.reciprocal(out=rden, in_=den)
        sc = work.tile([128, 256], BF16, tag="sc", name=f"sc{g}")
        nc.vector.tensor_scalar_mul(out=sc, in0=ex, scalar1=rden)
        sc_t[g] = sc

    def tail_tr(g):
        sc = sc_t[g]
        stc = []
        for jb in range(2):
            tp = psT.tile([128, 128], BF16, tag="tp", name=f"tp{g}{jb}")
            nc.tensor.transpose(tp, sc[:, 128 * jb:128 * jb + 128], ident)
            st = work.tile([128, 128], BF16, tag="stc", name=f"stc{g}{jb}")
            if jb == 0:
                nc.vector.tensor_copy(out=st, in_=tp)
            else:
                nc.scalar.copy(out=st, in_=tp)
            stc.append(st)
        stc_t[g] = stc

    def tail_av(g):
        stc = stc_t[g]
        av = psM.tile([128, 512], F32, tag="pm", name=f"av{g}")
        for h in range(2):
            hsl = slice(64 * h, 64 * h + 64)
            for jb in range(2):
                nc.tensor.matmul(av[hsl, 0:64], value[jb][:, hsl],
                                 stc[jb][:, hsl], start=(jb == 0),
                                 stop=(jb == 1), skip_group_check=True)
        aT = work.tile([128, 64], BF16, tag="aT", name=f"aT{g}")
        nc.scalar.copy(out=aT, in_=av[:, 0:64])
        aT_t[g] = aT

    def tail_out(g):
        aT = aT_t[g]
        ob = work.tile([64, 1024], F32, tag="ob", name=f"ob{g}")
        for eh in range(2):
            op = psM.tile([128, 512], F32, tag="pm", name=f"op{g}{eh}")
            nc.tensor.matmul(op[0:64, :], aT, wo[:, 512 * eh:512 * (eh + 1)],
                             start=True, stop=True, skip_group_check=True)
            if eh == 0:
                nc.vector.tensor_copy(out=ob[:, 0:512], in_=op[0:64, :])
            else:
                nc.scalar.copy(out=ob[:, 512:1024], in_=op[0:64, :])
        nc.scalar.dma_start(out=t["outp"][64 * g:64 * g + 64, :], in_=ob)

    # ---- schedule ----
    relT = const.tile([128, 256], BF16, tag="relT")
    nc.scalar.copy(out=relT, in_=proj_T(wr, xp, "ps_rel")[:, 0:256])
    a3_emit(relT)

    scatter_a(0)
    open_group(0)
    a2_run(0, 0, 16)
    scatter_b(0)
    a2_run(0, 16, 32)

    for jb in range(2):
        ps = psM.tile([128, 512], F32, tag="pm", name=f"ps_val{jb}")
        for c in range(8):
            nc.tensor.matmul(ps[:, 0:128], xv[:, c, 128 * jb:128 * jb + 128],
                             wv[:, c, :], start=(c == 0), stop=(c == 7))
        vsb = const.tile([128, 128], BF16, tag=f"value{jb}")
        nc.scalar.copy(out=vsb, in_=ps[:, 0:128])
        value[jb] = vsb

    a2_run(0, 32, 48)
    scatter_a(1)
    a2_run(0, 48, 64)
    a3_inject(0)
    tail_vec(0)

    open_group(1)
    a2_run(1, 0, 16)
    scatter_b(1)
    a2_run(1, 16, 32)
    tail_tr(0)
    scatter_a(2)
    a2_run(1, 32, 48)
    tail_av(0)
    a2_run(1, 48, 64)
    a3_inject(1)
    tail_vec(1)
    tail_out(0)

    for g in (2, 3):
        open_group(g)
        a2_run(g, 0, 16)
        scatter_b(g)
        a2_run(g, 16, 32)
        tail_tr(g - 1)
        if g < 3:
            scatter_a(g + 1)
        a2_run(g, 32, 48)
        tail_av(g - 1)
        a2_run(g, 48, 64)
        a3_inject(g)
        tail_vec(g)
        tail_out(g - 1)

    tail_tr(3)
    tail_av(3)
    tail_out(3)

    ctx.close()


def build():
    nc = bacc.Bacc("TRN2", target_bir_lowering=False, debug=False)
    t = {}

    def inp(name, shape, dt=BF16):
        t[name] = nc.dram_tensor(name, list(shape), dt,
                                 kind="ExternalInput").ap()

    inp("skT", (128, S * S))
    inp("sq", (128, 1), F32)
    for n in ("xq", "xk", "xp", "xv"):
        inp(n, (128, 8, 256))
    for n in ("wq", "wk", "wr", "wv"):
        inp(n, (128, 8, 128))
    inp("wo", (128, 1024))
    inp("mnot", (128, 4, 256))
    t["a3scr"] = nc.dram_tensor("a3scr", [4, 128, 512], BF16).ap()
    t["outp"] = nc.dram_tensor("outp", [S, E], F32, kind="ExternalOutput").ap()

    with tile.TileContext(nc) as tc:
        emit(tc, t)
    nc.compile()
    return nc


def make_in_maps(inputs: dict) -> list[dict]:
    q = np.asarray(inputs["q"], np.float32)
    k = np.asarray(inputs["k"], np.float32)
    v = np.asarray(inputs["v"], np.float32)
    p = np.asarray(inputs["p"], np.float32)
    mask = np.asarray(inputs["mask"])
    s_q = np.asarray(inputs["s_q"], np.float32)
    s_k = np.asarray(inputs["s_k"], np.float32)
    Wq = np.asarray(inputs["Wq"], np.float32)
    Wk = np.asarray(inputs["Wk"], np.float32)
    Wv = np.asarray(inputs["Wv"], np.float32)
    Wr = np.asarray(inputs["Wr"], np.float32)
    Wo = np.asarray(inputs["Wo"], np.float32)

    def actT(x):
        return np.ascontiguousarray(
            x.T.reshape(8, 128, 256).transpose(1, 0, 2)).astype(NPBF)

    xq, xk, xp, xv = actT(q), actT(k), actT(p), actT(v)

    maps = []
    for c in range(NCORES):
        rows = slice(c * DHB, (c + 1) * DHB)

        def wT(W):
            return np.ascontiguousarray(
                W[rows].T.reshape(8, 128, 128).transpose(1, 0, 2)
            ).astype(NPBF)

        skT = np.ascontiguousarray(s_k[:, rows].T).astype(NPBF)
        mn = np.empty((128, 4, 256), np.float32)
        for g in range(4):
            for h in range(2):
                mn[64 * h:64 * h + 64, g] = (
                    1.0 - mask[2 * c + h,
                               64 * g:64 * g + 64].astype(np.float32)
                ) * SCALING
        maps.append({
            "skT": skT,
            "xq": xq, "xk": xk, "xp": xp, "xv": xv,
            "wq": wT(Wq), "wk": wT(Wk), "wr": wT(Wr), "wv": wT(Wv),
            "wo": np.ascontiguousarray(Wo[:, rows].T).astype(NPBF),
            "sq": np.ascontiguousarray(s_q[0, rows][:, None]),
            "mnot": mn.astype(NPBF),
        })
    return maps


_NC = None


def kernel(**inputs) -> np.ndarray:
    global _NC
    from concourse.bass_utils import run_bass_kernel_spmd

    if _NC is None:
        _NC = build()
    maps = make_in_maps(inputs)
    res = run_bass_kernel_spmd(_NC, maps, list(range(NCORES)))
    out = np.zeros((S, E), np.float32)
    for r in res.results:
        out += np.asarray(r["outp"], np.float32)
    return out


if __name__ == "__main__":
    nc = build()
    print("build ok")
